# revision 1
# baseline (speedup 1.0000x reference)
"""Trainium2 Bass kernel for nn_DiagSSMBlock: h_t = tanh(a * h_{t-1} + (x @ b)_t).

Strategy (8 NeuronCores, H-sharded => zero cross-core communication):
  - Each core owns 256 of the 2048 channels (H axis). The diagonal recurrence
    is per-channel independent, so both the input projection GEMM and the scan
    are embarrassingly parallel across cores.
  - Host pre-transposes x -> xT [2048, 4096] so the GEMM runs as
    sT = bT.T @ xT with b stationary: the output lands directly in
    [channel, time] layout (channels on partitions, time on the free axis),
    which is what the scan needs. No on-device transposes.
  - The scan is solved by block Gauss-Seidel fixed-point iteration:
        u^0 = tanh(s);  u^m_t = tanh(a * u^{m-1}_{t-1} + s_t)
    Since |a| <= sqrt(2/2048) = 0.03125 (glorot init) and tanh is
    1-Lipschitz, each sweep contracts the error by |a|; after u0 + 2 sweeps
    the scan error is ~3e-5 absolute — well below the GEMM's fp16 input
    rounding (~9e-4 of output scale), which dominates the error budget.
    This turns the 4096-step serial recurrence into a handful of
    full-tensor elementwise passes (DVE scalar_tensor_tensor + ACT tanh),
    emitted in wavefront order so the scan overlaps the GEMM.
  - GEMM operands are cast to fp16 on host (x is N(0,1), b is glorot
    ~ +-0.031 — both comfortably in fp16 range): full-rate PE matmul and
    half the HBM traffic for x. Measured vs the fp32 reference:
    max|err| ~ 9.1e-4 of output scale, resid_var ~ 9e-8.
"""

import numpy as np

import jax
from jax.sharding import Mesh, NamedSharding, PartitionSpec
from jax.experimental.shard_map import shard_map

import concourse.tile as tile
from concourse import bacc, mybir
from concourse.bass2jax import (
    _bass_exec_p,
    install_neuronx_cc_hook,
    partition_id_tensor,
)

T = 4096          # sequence length
K = 2048          # input features (contraction dim)
N_CORES = 8
CPC = 256         # channels per core (H sharding)
NG = CPC // 128   # channel groups of 128 partitions per core
KT = K // 128     # k-tiles
TB = 512          # GEMM moving-dim block (one PSUM bank of fp32)
NSWEEPS = 2       # Gauss-Seidel refinement sweeps after u0 = tanh(s)
                  # (error contracts by |a|<=0.03125 per sweep: after u0 + 2
                  #  sweeps the scan error is ~3e-5, far below the GEMM's
                  #  fp16 rounding; the GEMM dominates the error budget)

F32 = mybir.dt.float32
GEMM_DT = mybir.dt.float16    # full-rate matmul; rel err ~9e-4 of scale


def _build(loop_iters: int, gemm_dt=GEMM_DT, nsweeps: int = NSWEEPS):
    nc = bacc.Bacc(
        "TRN2", target_bir_lowering=False, debug=False, num_devices=N_CORES
    )

    xt_d = nc.dram_tensor("xt", [K, T], gemm_dt, kind="ExternalInput").ap()
    bt_d = nc.dram_tensor("bt", [K, CPC], gemm_dt, kind="ExternalInput").ap()
    av_d = nc.dram_tensor("av", [128, NG], F32, kind="ExternalInput").ap()
    ht_d = nc.dram_tensor("ht", [CPC, T], F32, kind="ExternalOutput").ap()

    xt_r = xt_d.rearrange("(kt p) t -> p kt t", p=128)
    bt_r = bt_d.rearrange("(kt p) c -> p kt c", p=128)
    ht_r = ht_d.rearrange("(g p) t -> p g t", g=NG)

    Tanh = mybir.ActivationFunctionType.Tanh
    MUL = mybir.AluOpType.mult
    ADD = mybir.AluOpType.add

    with tile.TileContext(nc) as tc:
        with (
            tc.tile_pool(name="state", bufs=1) as state,
            tc.tile_pool(name="xp", bufs=7) as xpool,
            tc.tile_pool(name="ps", bufs=6, space="PSUM") as psum,
            tc.tile_pool(name="zp", bufs=4) as zpool,
        ):

            def body(_i):
                b_sb = state.tile([128, KT, CPC], gemm_dt, tag="b")
                a_sb = state.tile([128, NG], F32, tag="a")
                U = [state.tile([128, T + 1], F32, tag=f"U{g}", name=f"U{g}") for g in range(NG)]
                sT = [state.tile([128, T], F32, tag=f"sT{g}", name=f"sT{g}") for g in range(NG)]

                XKT = KT // 2  # k-tiles per x half-tile (finer DMA pipelining)
                NB = T // TB   # t-blocks

                # Startup: interleave the b stream with x-block-0 quarters so
                # the first matmuls (which need only b[kt<4] + a quarter of
                # x0) launch ~4us earlier; the DMA order is chosen so each
                # (kt, chunk) of block 0 lands just before PE consumes it.
                nc.sync.dma_start(out=b_sb[:, 0:4, :], in_=bt_r[:, 0:4, :])
                x0 = []
                QKT = KT // 4
                for q in range(4):
                    x_sb = xpool.tile(
                        [128, QKT, TB], gemm_dt, tag="xq", name=f"x_0_{q}"
                    )
                    x0.append(x_sb)
                nc.sync.dma_start(out=x0[0], in_=xt_r[:, 0:QKT, 0:TB])
                nc.sync.dma_start(out=x0[1], in_=xt_r[:, QKT:2 * QKT, 0:TB])
                nc.sync.dma_start(out=b_sb[:, 4:KT, :], in_=bt_r[:, 4:KT, :])
                nc.sync.dma_start(out=x0[2], in_=xt_r[:, 2 * QKT:3 * QKT, 0:TB])
                nc.sync.dma_start(out=x0[3], in_=xt_r[:, 3 * QKT:, 0:TB])
                nc.sync.dma_start(out=a_sb, in_=av_d)
                for g in range(NG):
                    nc.vector.memset(U[g][:, 0:1], 0.0)

                # --- Wavefront emission: GEMM block tb at wave tb; Gauss-
                # Seidel sweep m on block sb at wave m + sb. This interleaves
                # the per-engine static programs so the scan chases the GEMM
                # instead of serializing after it.
                def gemm_block(tb):
                    if tb == 0:
                        xs, ckt = x0, QKT
                    else:
                        xs = []
                        for h in range(2):
                            x_sb = xpool.tile(
                                [128, XKT, TB], gemm_dt, tag="x",
                                name=f"x_{tb}_{h}"
                            )
                            nc.sync.dma_start(
                                out=x_sb,
                                in_=xt_r[:, h * XKT:(h + 1) * XKT,
                                         tb * TB:(tb + 1) * TB],
                            )
                            xs.append(x_sb)
                        ckt = XKT
                    for g in range(NG):
                        ps = psum.tile([128, TB], F32, tag="ps")
                        for kt in range(KT):
                            nc.tensor.matmul(
                                ps,
                                lhsT=b_sb[:, kt, g * 128:(g + 1) * 128],
                                rhs=xs[kt // ckt][:, kt % ckt, :],
                                start=(kt == 0),
                                stop=(kt == KT - 1),
                            )
                        dst = sT[g][:, tb * TB:(tb + 1) * TB]
                        nc.vector.tensor_copy(out=dst, in_=ps)
                        nc.scalar.activation(
                            out=U[g][:, 1 + tb * TB: 1 + (tb + 1) * TB],
                            in_=ps,
                            func=Tanh,
                        )

                def sweep_range(m, lo, hi, tag):
                    last = m == nsweeps - 1
                    for g in range(NG):
                        z = zpool.tile(
                            [128, hi - lo], F32, tag="z", name=f"z_{tag}_{g}"
                        )
                        nc.vector.scalar_tensor_tensor(
                            out=z,
                            in0=U[g][:, lo:hi],
                            scalar=a_sb[:, g:g + 1],
                            in1=sT[g][:, lo:hi],
                            op0=MUL,
                            op1=ADD,
                        )
                        nc.scalar.activation(
                            out=U[g][:, 1 + lo: 1 + hi], in_=z, func=Tanh
                        )
                        if last:
                            nc.scalar.dma_start(
                                out=ht_r[:, g, lo:hi],
                                in_=U[g][:, 1 + lo: 1 + hi],
                            )

                def sweep_block(m, sb):
                    sweep_range(m, sb * TB, (sb + 1) * TB, f"{m}_{sb}")

                for wave in range(NB + nsweeps):
                    if wave < NB:
                        gemm_block(wave)
                    for m in range(1, nsweeps + 1):
                        sb = wave - m
                        if 0 <= sb < NB:
                            sweep_block(m - 1, sb)

            if loop_iters == 1:
                body(0)
            else:
                with tc.For_i(
                    0, loop_iters, 1, hint_engines=(mybir.EngineType.PE,)
                ) as i:
                    body(i)

    nc.compile()
    return nc


def _build_runner(nc):
    """Reusable jitted shard_map executable for an 8-core SPMD Bass module."""
    install_neuronx_cc_hook()
    partition_name = nc.partition_id_tensor.name if nc.partition_id_tensor else None
    in_names, out_names, out_avals = [], [], []
    for alloc in nc.m.functions[0].allocations:
        if not isinstance(alloc, mybir.MemoryLocationSet):
            continue
        name = alloc.memorylocations[0].name
        if alloc.kind == "ExternalInput":
            if name != partition_name:
                in_names.append(name)
        elif alloc.kind == "ExternalOutput":
            out_names.append(name)
            out_avals.append(
                jax.core.ShapedArray(
                    tuple(alloc.tensor_shape), mybir.dt.np(alloc.dtype)
                )
            )
    n_params = len(in_names)
    n_outs = len(out_avals)
    in_names_all = list(in_names) + list(out_names)
    if partition_name is not None:
        in_names_all.append(partition_name)
    donate = tuple(range(n_params, n_params + n_outs))

    def _bdy(*args):
        operands = list(args)
        if partition_name is not None:
            operands.append(partition_id_tensor())
        return tuple(
            _bass_exec_p.bind(
                *operands,
                out_avals=tuple(out_avals),
                in_names=tuple(in_names_all),
                out_names=tuple(out_names),
                lowering_input_output_aliases=(),
                sim_require_finite=True,
                sim_require_nnan=True,
                nc=nc,
            )
        )

    devices = jax.devices()[:N_CORES]
    mesh = Mesh(np.asarray(devices), ("core",))
    in_specs = (PartitionSpec("core"),) * (n_params + n_outs)
    out_specs = (PartitionSpec("core"),) * len(out_names)
    sharded = jax.jit(
        shard_map(
            _bdy, mesh=mesh, in_specs=in_specs, out_specs=out_specs,
            check_rep=False,
        ),
        donate_argnums=donate,
        keep_unused=True,
    )
    shardng = NamedSharding(mesh, PartitionSpec("core"))
    out_shapes = [
        (N_CORES * a.shape[0], *a.shape[1:]) for a in out_avals
    ]
    out_dtypes = [a.dtype for a in out_avals]

    class Runner:
        def put_inputs(self, in_maps):
            concat = [
                np.concatenate([m[n] for m in in_maps], axis=0) for n in in_names
            ]
            return [jax.device_put(a, shardng) for a in concat]

        def zeros(self):
            return [
                jax.device_put(np.zeros(s, d), shardng)
                for s, d in zip(out_shapes, out_dtypes)
            ]

        def exec_device(self, dev_in, dev_zeros):
            outs = sharded(*dev_in, *dev_zeros)
            jax.block_until_ready(outs)
            return outs

        def fetch(self, outs):
            return {
                name: np.asarray(outs[i]).reshape(N_CORES, -1, *out_avals[i].shape[1:])
                for i, name in enumerate(out_names)
            }

        def __call__(self, dev_in, dev_zeros):
            return self.fetch(self.exec_device(dev_in, dev_zeros))

    return Runner()


_CACHE: dict = {}


def get_compiled(loop_iters=1, gemm_dt=GEMM_DT, nsweeps=NSWEEPS):
    key = (loop_iters, str(gemm_dt), nsweeps)
    if key not in _CACHE:
        nc = _build(loop_iters, gemm_dt, nsweeps)
        _CACHE[key] = (nc, _build_runner(nc))
    return _CACHE[key]


def make_in_maps(x, a_mat, b_mat, gemm_dt=GEMM_DT):
    gemm_np = mybir.dt.np(gemm_dt)
    x = np.ascontiguousarray(np.asarray(x, np.float32))
    a_mat = np.ascontiguousarray(np.asarray(a_mat, np.float32))
    b_mat = np.ascontiguousarray(np.asarray(b_mat, np.float32))
    xt = np.ascontiguousarray(x.T).astype(gemm_np)  # [K, T]
    bm = b_mat.astype(gemm_np)
    in_maps = []
    for c in range(N_CORES):
        sl = slice(c * CPC, (c + 1) * CPC)
        in_maps.append(
            {
                "xt": xt,
                "bt": np.ascontiguousarray(bm[:, sl]),
                "av": np.ascontiguousarray(a_mat[sl].reshape(NG, 128).T),
            }
        )
    return in_maps


def kernel(x, a_mat, b_mat):
    from concourse import bass_utils

    key = ("nc1", str(GEMM_DT), NSWEEPS)
    if key not in _CACHE:
        _CACHE[key] = _build(1, GEMM_DT, NSWEEPS)
    nc = _CACHE[key]
    in_maps = make_in_maps(x, a_mat, b_mat)
    res = bass_utils.run_bass_kernel_spmd(nc, in_maps, core_ids=list(range(N_CORES)))
    ht = np.concatenate(
        [np.asarray(res.results[c]["ht"]) for c in range(N_CORES)], axis=0
    )  # [H, T]: cores stacked along the channel axis
    return np.ascontiguousarray(ht.T).astype(np.float32)  # [T, H]



# revision 2
# speedup vs baseline: 1.0112x; 1.0112x over previous
"""Trainium2 Bass kernel for nn_DiagSSMBlock: h_t = tanh(a * h_{t-1} + (x @ b)_t).

Strategy (8 NeuronCores, T x H = 2 x 4 sharding => zero cross-core comm):
  - Cores are arranged 2-way along time (chunks of 2048 steps) x 4-way along
    channels (512 each). The diagonal recurrence is per-channel independent,
    and |a| <= sqrt(2/2048) = 0.03125 (glorot) means a time-chunk started
    from a zero carry converges to the true scan within a few steps: each
    chunk is given W=16 warm-up columns (the tail of the previous chunk;
    zeros for the first chunk), so the boundary error is ~|a|^17 ~ 0.
  - This cuts per-core HBM traffic from 21 MiB (H-only sharding: every core
    reads all of x) to ~14 MiB, moving the kernel from DMA-bound to the
    fp16 matmul roofline (4g*16kt*2064col = 132k PE cycles ~ 55us @2.4GHz).
  - GEMM: sT = b^T x with b stationary; host pre-swizzles x and b into the
    exact SBUF layout ([128 part, kt, t] with 16 KiB contiguous per
    partition) so every DMA descriptor is a 16 KiB run (near-peak DMA).
  - The scan is solved by block Gauss-Seidel fixed-point iteration:
        u^0 = tanh(s);  u^m_t = tanh(a * u^{m-1}_{t-1} + s_t)
    Each sweep contracts the error by |a| <= 0.03125; after u0 + 2 sweeps
    the scan error (~3e-5) is far below the fp16 GEMM rounding (~9e-4 of
    output scale), which dominates the error budget. Sweeps are emitted in
    wavefront order so the scan (DVE+ACT) chases the GEMM (PE).
  - Output DMA is issued from the otherwise-idle Pool/GpSimd queue so the
    ACT and SP sequencers stay free for the scan and input prefetch.
"""

import numpy as np

import jax
from jax.sharding import Mesh, NamedSharding, PartitionSpec
from jax.experimental.shard_map import shard_map

import concourse.tile as tile
from concourse import bacc, mybir
from concourse.bass2jax import (
    _bass_exec_p,
    install_neuronx_cc_hook,
    partition_id_tensor,
)

T = 4096          # sequence length
K = 2048          # input features (contraction dim)
H = 2048          # output channels
N_CORES = 8
PT = 2            # T-shards
QH = 4            # H-shards
T_LOC = T // PT   # timesteps per core (graded)
W = 16            # warm-up columns (zero-carry decay: |a|^17 ~ 1e-25)
T_TOT = T_LOC + W
CPC = H // QH     # channels per core
NG = CPC // 128   # channel groups of 128 partitions
KT = K // 128     # k-tiles
TB = 512          # GEMM moving-dim block (one PSUM bank of fp32)
NB = T_LOC // TB  # main t-blocks per core
NSWEEPS = 2       # Gauss-Seidel refinement sweeps after u0 = tanh(s)

F32 = mybir.dt.float32
GEMM_DT = mybir.dt.float16    # full-rate matmul; rel err ~9e-4 of scale


def _build(loop_iters: int, gemm_dt=GEMM_DT, nsweeps: int = NSWEEPS):
    nc = bacc.Bacc(
        "TRN2", target_bir_lowering=False, debug=False, num_devices=N_CORES
    )

    xb_d = nc.dram_tensor("xb", [NB, 128, KT, TB], gemm_dt, kind="ExternalInput").ap()
    xw_d = nc.dram_tensor("xw", [128, KT, W], gemm_dt, kind="ExternalInput").ap()
    bt_d = nc.dram_tensor("bt", [128, KT, CPC], gemm_dt, kind="ExternalInput").ap()
    av_d = nc.dram_tensor("av", [128, NG], F32, kind="ExternalInput").ap()
    ht_d = nc.dram_tensor("ht", [CPC, T_LOC], F32, kind="ExternalOutput").ap()

    ht_r = ht_d.rearrange("(g p) t -> p g t", g=NG)

    Tanh = mybir.ActivationFunctionType.Tanh
    MUL = mybir.AluOpType.mult
    ADD = mybir.AluOpType.add

    # ranges in warm-up-inclusive column space [0, T_TOT)
    ranges = [(0, W)] + [(W + i * TB, W + (i + 1) * TB) for i in range(NB)]
    NR = len(ranges)

    with tile.TileContext(nc) as tc:
        with (
            tc.tile_pool(name="state", bufs=1) as state,
            tc.tile_pool(name="bp", bufs=2) as bpool,
            tc.tile_pool(name="xp", bufs=5) as xpool,
            tc.tile_pool(name="ps", bufs=6, space="PSUM") as psum,
            tc.tile_pool(name="zp", bufs=4) as zpool,
        ):

            def body(_i):
                b_sb = bpool.tile([128, KT, CPC], gemm_dt, tag="b")
                a_sb = state.tile([128, NG], F32, tag="a")
                U = [state.tile([128, T_TOT + 1], F32, tag=f"U{g}", name=f"U{g}")
                     for g in range(NG)]
                sT = [state.tile([128, T_TOT], F32, tag=f"sT{g}", name=f"sT{g}")
                      for g in range(NG)]

                # input prefetch: b first (all blocks need it), then the
                # warm-up columns and the main x blocks in consumption order.
                nc.sync.dma_start(out=b_sb, in_=bt_d)
                xw_sb = xpool.tile([128, KT, W], gemm_dt, tag="xw", name="xw")
                nc.sync.dma_start(out=xw_sb, in_=xw_d)
                xs = []
                for nb in range(NB):
                    x_sb = xpool.tile([128, KT, TB], gemm_dt, tag="xb",
                                      name=f"x_{nb}")
                    nc.sync.dma_start(out=x_sb, in_=xb_d[nb])
                    xs.append(x_sb)
                nc.sync.dma_start(out=a_sb, in_=av_d)
                for g in range(NG):
                    nc.vector.memset(U[g][:, 0:1], 0.0)

                # --- wavefront emission: GEMM range r at wave r; sweep m on
                # range r at wave r + m, so the scan chases the GEMM.
                def gemm_range(ri):
                    lo, hi = ranges[ri]
                    rhs = xw_sb if ri == 0 else xs[ri - 1]
                    for g in range(NG):
                        ps = psum.tile([128, hi - lo], F32, tag="ps")
                        for kt in range(KT):
                            nc.tensor.matmul(
                                ps,
                                lhsT=b_sb[:, kt, g * 128:(g + 1) * 128],
                                rhs=rhs[:, kt, :],
                                start=(kt == 0),
                                stop=(kt == KT - 1),
                            )
                        nc.vector.tensor_copy(out=sT[g][:, lo:hi], in_=ps)
                        nc.scalar.activation(
                            out=U[g][:, 1 + lo:1 + hi], in_=ps, func=Tanh
                        )

                def sweep_range(m, ri):
                    lo, hi = ranges[ri]
                    last = m == nsweeps - 1
                    for g in range(NG):
                        z = zpool.tile(
                            [128, hi - lo], F32, tag="z", name=f"z_{m}_{ri}_{g}"
                        )
                        nc.vector.scalar_tensor_tensor(
                            out=z,
                            in0=U[g][:, lo:hi],
                            scalar=a_sb[:, g:g + 1],
                            in1=sT[g][:, lo:hi],
                            op0=MUL,
                            op1=ADD,
                        )
                        nc.scalar.activation(
                            out=U[g][:, 1 + lo:1 + hi], in_=z, func=Tanh
                        )
                        if last and ri > 0:
                            nc.gpsimd.dma_start(
                                out=ht_r[:, g, lo - W:hi - W],
                                in_=U[g][:, 1 + lo:1 + hi],
                            )

                for wave in range(NR + nsweeps):
                    if wave < NR:
                        gemm_range(wave)
                    for m in range(1, nsweeps + 1):
                        ri = wave - m
                        if 0 <= ri < NR:
                            sweep_range(m - 1, ri)

            if loop_iters == 1:
                body(0)
            else:
                with tc.For_i(
                    0, loop_iters, 1, hint_engines=(mybir.EngineType.PE,)
                ) as i:
                    body(i)

    nc.compile()
    return nc


def _build_runner(nc):
    """Reusable jitted shard_map executable for an 8-core SPMD Bass module."""
    install_neuronx_cc_hook()
    partition_name = nc.partition_id_tensor.name if nc.partition_id_tensor else None
    in_names, out_names, out_avals = [], [], []
    for alloc in nc.m.functions[0].allocations:
        if not isinstance(alloc, mybir.MemoryLocationSet):
            continue
        name = alloc.memorylocations[0].name
        if alloc.kind == "ExternalInput":
            if name != partition_name:
                in_names.append(name)
        elif alloc.kind == "ExternalOutput":
            out_names.append(name)
            out_avals.append(
                jax.core.ShapedArray(
                    tuple(alloc.tensor_shape), mybir.dt.np(alloc.dtype)
                )
            )
    n_params = len(in_names)
    n_outs = len(out_avals)
    in_names_all = list(in_names) + list(out_names)
    if partition_name is not None:
        in_names_all.append(partition_name)
    donate = tuple(range(n_params, n_params + n_outs))

    def _bdy(*args):
        operands = list(args)
        if partition_name is not None:
            operands.append(partition_id_tensor())
        return tuple(
            _bass_exec_p.bind(
                *operands,
                out_avals=tuple(out_avals),
                in_names=tuple(in_names_all),
                out_names=tuple(out_names),
                lowering_input_output_aliases=(),
                sim_require_finite=True,
                sim_require_nnan=True,
                nc=nc,
            )
        )

    devices = jax.devices()[:N_CORES]
    mesh = Mesh(np.asarray(devices), ("core",))
    in_specs = (PartitionSpec("core"),) * (n_params + n_outs)
    out_specs = (PartitionSpec("core"),) * len(out_names)
    sharded = jax.jit(
        shard_map(
            _bdy, mesh=mesh, in_specs=in_specs, out_specs=out_specs,
            check_rep=False,
        ),
        donate_argnums=donate,
        keep_unused=True,
    )
    shardng = NamedSharding(mesh, PartitionSpec("core"))
    out_shapes = [
        (N_CORES * a.shape[0], *a.shape[1:]) for a in out_avals
    ]
    out_dtypes = [a.dtype for a in out_avals]

    class Runner:
        def put_inputs(self, in_maps):
            concat = [
                np.concatenate([m[n] for m in in_maps], axis=0) for n in in_names
            ]
            return [jax.device_put(a, shardng) for a in concat]

        def zeros(self):
            return [
                jax.device_put(np.zeros(s, d), shardng)
                for s, d in zip(out_shapes, out_dtypes)
            ]

        def exec_device(self, dev_in, dev_zeros):
            outs = sharded(*dev_in, *dev_zeros)
            jax.block_until_ready(outs)
            return outs

        def fetch(self, outs):
            return {
                name: np.asarray(outs[i]).reshape(N_CORES, -1, *out_avals[i].shape[1:])
                for i, name in enumerate(out_names)
            }

        def __call__(self, dev_in, dev_zeros):
            return self.fetch(self.exec_device(dev_in, dev_zeros))

    return Runner()


_CACHE: dict = {}


def get_compiled(loop_iters=1, gemm_dt=GEMM_DT, nsweeps=NSWEEPS):
    key = (loop_iters, str(gemm_dt), nsweeps)
    if key not in _CACHE:
        nc = _build(loop_iters, gemm_dt, nsweeps)
        _CACHE[key] = (nc, _build_runner(nc))
    return _CACHE[key]


def make_in_maps(x, a_mat, b_mat, gemm_dt=GEMM_DT):
    gemm_np = mybir.dt.np(gemm_dt)
    x = np.ascontiguousarray(np.asarray(x, np.float32))
    a_mat = np.ascontiguousarray(np.asarray(a_mat, np.float32))
    b_mat = np.ascontiguousarray(np.asarray(b_mat, np.float32))
    in_maps = []
    for c in range(N_CORES):
        p, q = divmod(c, QH)
        t0 = p * T_LOC
        qs = slice(q * CPC, (q + 1) * CPC)
        # b: [k, c] -> [p, kt, c] with k = kt*128 + p
        b_sw = np.ascontiguousarray(
            b_mat[:, qs].reshape(KT, 128, CPC).transpose(1, 0, 2)
        ).astype(gemm_np)
        # x main blocks: [t, k] -> [nb, p, kt, tt], t = t0 + nb*TB + tt
        xt = x[t0:t0 + T_LOC].T.astype(gemm_np)          # [K, T_LOC]
        xb = np.ascontiguousarray(
            xt.reshape(KT, 128, NB, TB).transpose(2, 1, 0, 3)
        )
        # warm-up columns: previous chunk's tail (zeros for the first chunk)
        if p == 0:
            xw = np.zeros((128, KT, W), gemm_np)
        else:
            xw = np.ascontiguousarray(
                x[t0 - W:t0].T.astype(gemm_np).reshape(KT, 128, W)
                .transpose(1, 0, 2)
            )
        av = np.ascontiguousarray(a_mat[qs].reshape(NG, 128).T)
        in_maps.append({"xb": xb, "xw": xw, "bt": b_sw, "av": av})
    return in_maps


def kernel(x, a_mat, b_mat):
    from concourse import bass_utils

    key = ("nc1", str(GEMM_DT), NSWEEPS)
    if key not in _CACHE:
        _CACHE[key] = _build(1, GEMM_DT, NSWEEPS)
    nc = _CACHE[key]
    in_maps = make_in_maps(x, a_mat, b_mat)
    res = bass_utils.run_bass_kernel_spmd(nc, in_maps, core_ids=list(range(N_CORES)))
    out = np.empty((T, H), np.float32)
    for c in range(N_CORES):
        p, q = divmod(c, QH)
        ht = np.asarray(res.results[c]["ht"])   # [CPC, T_LOC]
        out[p * T_LOC:(p + 1) * T_LOC, q * CPC:(q + 1) * CPC] = ht.T
    return out


# revision 19
# speedup vs baseline: 1.2659x; 1.2518x over previous
"""Trainium2 Bass kernel for nn_DiagSSMBlock: h_t = tanh(a * h_{t-1} + (x @ b)_t).

Strategy (8 NeuronCores, T x H = 2 x 4 sharding => zero cross-core comm):
  - Cores are arranged 2-way along time (chunks of 2048 steps) x 4-way along
    channels (512 each). The diagonal recurrence is per-channel independent,
    and |a| <= sqrt(2/2048) = 0.03125 (glorot) means a time-chunk started
    from a zero carry converges to the true scan within a few steps: each
    chunk is given W=16 warm-up columns (the tail of the previous chunk;
    zeros for the first chunk), so the boundary error is ~|a|^17 ~ 0.
  - This cuts per-core HBM traffic from 21 MiB (H-only sharding: every core
    reads all of x) to ~14 MiB, moving the kernel from DMA-bound to the
    fp16 matmul roofline (4g*16kt*2064col = 132k PE cycles ~ 55us @2.4GHz).
  - GEMM: sT = b^T x with b stationary; host pre-swizzles x and b into the
    exact SBUF layout ([128 part, kt, t] with 16 KiB contiguous per
    partition) so every DMA descriptor is a 16 KiB run (near-peak DMA).
  - The scan is solved by block Gauss-Seidel fixed-point iteration:
        u^0 = tanh(s);  u^m_t = tanh(a * u^{m-1}_{t-1} + s_t)
    Each sweep contracts the error by |a| <= 0.03125; after u0 + 2 sweeps
    the scan error (~3e-5) is far below the fp16 GEMM rounding (~9e-4 of
    output scale), which dominates the error budget. Sweeps are emitted in
    wavefront order so the scan (DVE+ACT) chases the GEMM (PE).
  - Output DMA is issued from the otherwise-idle Pool/GpSimd queue so the
    ACT and SP sequencers stay free for the scan and input prefetch.
"""

import numpy as np

import jax
from jax.sharding import Mesh, NamedSharding, PartitionSpec
from jax.experimental.shard_map import shard_map

import concourse.tile as tile
from concourse import bacc, mybir
from concourse.bass2jax import (
    _bass_exec_p,
    install_neuronx_cc_hook,
    partition_id_tensor,
)

T = 4096          # sequence length
K = 2048          # input features (contraction dim)
H = 2048          # output channels
N_CORES = 8
PT = 2            # T-shards
QH = 4            # H-shards
T_LOC = T // PT   # timesteps per core (graded)
W = 16            # warm-up columns (zero-carry decay: |a|^17 ~ 1e-25)
T_TOT = T_LOC + W
CPC = H // QH     # channels per core
NG = CPC // 128   # channel groups of 128 partitions
KT = K // 128     # k-tiles
TB = 512          # GEMM moving-dim block (one PSUM bank of fp32)
NB = T_LOC // TB  # main t-blocks per core
NSWEEPS = 1       # Gauss-Seidel refinement sweeps after u0 = tanh(s)
                  # (|a|<=0.03125: worst-case scan err a^2 ~ 9.8e-4, on par
                  #  with the fp16 GEMM rounding; total well under the gate)

F32 = mybir.dt.float32
GEMM_DT = mybir.dt.float16    # full-rate matmul; rel err ~9e-4 of scale


UNROLL = 4          # bodies per For_i trip (input buffers alternate via pools)
STAGGERED = False   # staggered sem reset at the back-edge (no full barrier)
OUT_ENGINE = "gpsimd"  # queue for output DMAs


def _build(loop_iters: int, gemm_dt=GEMM_DT, nsweeps: int = NSWEEPS,
           unroll: int = None, staggered: bool = None):
    if unroll is None:
        unroll = UNROLL
    if staggered is None:
        staggered = STAGGERED
    nc = bacc.Bacc(
        "TRN2", target_bir_lowering=False, debug=False, num_devices=N_CORES
    )

    xb_d = nc.dram_tensor("xb", [NB, 128, KT, TB], gemm_dt, kind="ExternalInput").ap()
    xw_d = nc.dram_tensor("xw", [128, KT, W], gemm_dt, kind="ExternalInput").ap()
    bt_d = nc.dram_tensor("bt", [128, KT, CPC], gemm_dt, kind="ExternalInput").ap()
    av_d = nc.dram_tensor("av", [128, NG], F32, kind="ExternalInput").ap()
    ht_d = nc.dram_tensor("ht", [CPC, T_LOC], F32, kind="ExternalOutput").ap()

    ht_r = ht_d.rearrange("(g p) t -> p g t", g=NG)

    # ranges in warm-up-inclusive column space [0, T_TOT); the tail is
    # tapered so the trailing sweep+output pipeline after the last matmul
    # drains on small blocks (shorter For_i barrier tail)
    widths = [W, TB, TB, TB, TB // 2, TB // 4, TB // 4]
    assert sum(widths) == T_TOT
    ranges, pos = [], 0
    for wd in widths:
        ranges.append((pos, pos + wd))
        pos += wd
    NR = len(ranges)

    with tile.TileContext(nc) as tc:
        with (
            tc.tile_pool(name="state", bufs=1) as state,
            tc.tile_pool(name="bp", bufs=2) as bpool,
            tc.tile_pool(name="xp", bufs=1) as xpool,
            tc.tile_pool(name="ap", bufs=2) as apool,
            tc.tile_pool(name="ps", bufs=6, space="PSUM") as psum,
            tc.tile_pool(name="zp", bufs=4) as zpool,
        ):
            # Fixed tile objects. b/a ping-pong across bodies (parity); the
            # x tiles are refilled in place right after the wave that
            # consumes them, one body ahead of their next use, so every
            # body's inputs are resident when its matmuls start (the For_i
            # back-edge barrier then never exposes DMA latency).
            b_tiles = [bpool.tile([128, KT, CPC], gemm_dt, tag="b",
                                  name=f"b{j}") for j in range(2)]
            a_tiles = [apool.tile([128, NG], F32, tag="a", name=f"a{j}")
                       for j in range(2)]
            xs = [xpool.tile([128, KT, W], gemm_dt, tag="xw", name="xw")]
            xs += [xpool.tile([128, KT, TB], gemm_dt, tag=f"x{nb}",
                              name=f"x{nb}") for nb in range(NB - 1)]
            # the last block is ping-ponged by body parity: its refill for
            # the next body has no WAR left by body top, so its DMA never
            # lands on the trip tail (where it would gate the barrier)
            xl_tiles = [xpool.tile([128, KT, TB], gemm_dt, tag=f"xl{j}",
                                   name=f"xl{j}") for j in range(2)]
            x_src = [xw_d] + [xb_d[nb] for nb in range(NB)]
            U = [state.tile([128, T_TOT + 1], F32, tag=f"U{g}", name=f"U{g}")
                 for g in range(NG)]
            sT = [state.tile([128, T_TOT], F32, tag=f"sT{g}", name=f"sT{g}")
                  for g in range(NG)]

            Tanh = mybir.ActivationFunctionType.Tanh
            MUL = mybir.AluOpType.mult
            ADD = mybir.AluOpType.add

            def prologue():
                nc.sync.dma_start(out=b_tiles[0], in_=bt_d)
                nc.sync.dma_start(out=a_tiles[0], in_=av_d)
                for j in range(len(xs)):
                    nc.sync.dma_start(out=xs[j], in_=x_src[j])
                nc.sync.dma_start(out=xl_tiles[0], in_=xb_d[NB - 1])

            def body(it, parity, load_next):
                b_sb = b_tiles[parity]
                a_sb = a_tiles[parity]
                # range ri -> (sbuf tile, column offset within the tile)
                xl = xl_tiles[parity]
                rmap = []
                for ri, (lo, hi) in enumerate(ranges):
                    if lo < W + (NB - 1) * TB:
                        rmap.append((xs[ri], 0))      # 1:1 tile per range
                    else:
                        rmap.append((xl, lo - (W + (NB - 1) * TB)))
                if load_next:
                    nc.sync.dma_start(out=b_tiles[1 - parity], in_=bt_d)
                    nc.sync.dma_start(out=xl_tiles[1 - parity],
                                      in_=xb_d[NB - 1])
                    nc.sync.dma_start(out=a_tiles[1 - parity], in_=av_d)
                for g in range(NG):
                    nc.vector.memset(U[g][:, 0:1], 0.0)

                def gemm_range(ri):
                    lo, hi = ranges[ri]
                    xt, xo = rmap[ri]
                    for g in range(NG):
                        ps = psum.tile([128, hi - lo], F32, tag="ps")
                        for kt in range(KT):
                            nc.tensor.matmul(
                                ps,
                                lhsT=b_sb[:, kt, g * 128:(g + 1) * 128],
                                rhs=xt[:, kt, xo:xo + (hi - lo)],
                                start=(kt == 0),
                                stop=(kt == KT - 1),
                            )
                        nc.vector.tensor_copy(out=sT[g][:, lo:hi], in_=ps)
                        nc.scalar.activation(
                            out=U[g][:, 1 + lo:1 + hi], in_=ps, func=Tanh
                        )
                    if load_next and ri < NR and rmap[ri][0] is not xl and ri < len(x_src):
                        # refill this block for the next body (consumed
                        # above; next body's wave ri is ~5 waves away)
                        nc.sync.dma_start(out=xs[ri], in_=x_src[ri])

                def sweep_range(m, ri):
                    lo, hi = ranges[ri]
                    last = m == nsweeps - 1
                    for g in range(NG):
                        z = zpool.tile(
                            [128, hi - lo], F32, tag="z",
                            name=f"z_{it}_{m}_{ri}_{g}"
                        )
                        nc.vector.scalar_tensor_tensor(
                            out=z,
                            in0=U[g][:, lo:hi],
                            scalar=a_sb[:, g:g + 1],
                            in1=sT[g][:, lo:hi],
                            op0=MUL,
                            op1=ADD,
                        )
                        nc.scalar.activation(
                            out=U[g][:, 1 + lo:1 + hi], in_=z, func=Tanh
                        )
                        if last and ri > 0:
                            getattr(nc, OUT_ENGINE).dma_start(
                                out=ht_r[:, g, lo - W:hi - W],
                                in_=U[g][:, 1 + lo:1 + hi],
                            )

                for wave in range(NR + nsweeps):
                    if wave < NR:
                        gemm_range(wave)
                    for m in range(1, nsweeps + 1):
                        ri = wave - m
                        if 0 <= ri < NR:
                            sweep_range(m - 1, ri)

            assert unroll % 2 == 0
            trips, rem = divmod(loop_iters, unroll)
            prologue()
            if trips <= 1:
                n = loop_iters
                for u in range(n):
                    body(f"i{u}", u % 2, u < n - 1)
            else:
                with tc.For_i(
                    0, trips, 1, hint_engines=(mybir.EngineType.PE,),
                    staggered_reset=staggered,
                ) as i:
                    for u in range(unroll):
                        body(f"u{u}", u % 2, True)
                for r in range(rem):
                    body(f"r{r}", r % 2, r < rem - 1)

    nc.compile()
    return nc


def _build_runner(nc):
    """Reusable jitted shard_map executable for an 8-core SPMD Bass module."""
    install_neuronx_cc_hook()
    partition_name = nc.partition_id_tensor.name if nc.partition_id_tensor else None
    in_names, out_names, out_avals = [], [], []
    for alloc in nc.m.functions[0].allocations:
        if not isinstance(alloc, mybir.MemoryLocationSet):
            continue
        name = alloc.memorylocations[0].name
        if alloc.kind == "ExternalInput":
            if name != partition_name:
                in_names.append(name)
        elif alloc.kind == "ExternalOutput":
            out_names.append(name)
            out_avals.append(
                jax.core.ShapedArray(
                    tuple(alloc.tensor_shape), mybir.dt.np(alloc.dtype)
                )
            )
    n_params = len(in_names)
    n_outs = len(out_avals)
    in_names_all = list(in_names) + list(out_names)
    if partition_name is not None:
        in_names_all.append(partition_name)
    donate = tuple(range(n_params, n_params + n_outs))

    def _bdy(*args):
        operands = list(args)
        if partition_name is not None:
            operands.append(partition_id_tensor())
        return tuple(
            _bass_exec_p.bind(
                *operands,
                out_avals=tuple(out_avals),
                in_names=tuple(in_names_all),
                out_names=tuple(out_names),
                lowering_input_output_aliases=(),
                sim_require_finite=True,
                sim_require_nnan=True,
                nc=nc,
            )
        )

    devices = jax.devices()[:N_CORES]
    mesh = Mesh(np.asarray(devices), ("core",))
    in_specs = (PartitionSpec("core"),) * (n_params + n_outs)
    out_specs = (PartitionSpec("core"),) * len(out_names)
    sharded = jax.jit(
        shard_map(
            _bdy, mesh=mesh, in_specs=in_specs, out_specs=out_specs,
            check_rep=False,
        ),
        donate_argnums=donate,
        keep_unused=True,
    )
    shardng = NamedSharding(mesh, PartitionSpec("core"))
    out_shapes = [
        (N_CORES * a.shape[0], *a.shape[1:]) for a in out_avals
    ]
    out_dtypes = [a.dtype for a in out_avals]

    class Runner:
        def put_inputs(self, in_maps):
            concat = [
                np.concatenate([m[n] for m in in_maps], axis=0) for n in in_names
            ]
            return [jax.device_put(a, shardng) for a in concat]

        def zeros(self):
            return [
                jax.device_put(np.zeros(s, d), shardng)
                for s, d in zip(out_shapes, out_dtypes)
            ]

        def exec_device(self, dev_in, dev_zeros):
            outs = sharded(*dev_in, *dev_zeros)
            jax.block_until_ready(outs)
            return outs

        def fetch(self, outs):
            return {
                name: np.asarray(outs[i]).reshape(N_CORES, -1, *out_avals[i].shape[1:])
                for i, name in enumerate(out_names)
            }

        def __call__(self, dev_in, dev_zeros):
            return self.fetch(self.exec_device(dev_in, dev_zeros))

    return Runner()


_CACHE: dict = {}


def get_compiled(loop_iters=1, gemm_dt=GEMM_DT, nsweeps=NSWEEPS):
    key = (loop_iters, str(gemm_dt), nsweeps)
    if key not in _CACHE:
        nc = _build(loop_iters, gemm_dt, nsweeps)
        _CACHE[key] = (nc, _build_runner(nc))
    return _CACHE[key]


def make_in_maps(x, a_mat, b_mat, gemm_dt=GEMM_DT):
    gemm_np = mybir.dt.np(gemm_dt)
    x = np.ascontiguousarray(np.asarray(x, np.float32))
    a_mat = np.ascontiguousarray(np.asarray(a_mat, np.float32))
    b_mat = np.ascontiguousarray(np.asarray(b_mat, np.float32))
    in_maps = []
    for c in range(N_CORES):
        p, q = divmod(c, QH)
        t0 = p * T_LOC
        qs = slice(q * CPC, (q + 1) * CPC)
        # b: [k, c] -> [p, kt, c] with k = kt*128 + p
        b_sw = np.ascontiguousarray(
            b_mat[:, qs].reshape(KT, 128, CPC).transpose(1, 0, 2)
        ).astype(gemm_np)
        # x main blocks: [t, k] -> [nb, p, kt, tt], t = t0 + nb*TB + tt
        xt = x[t0:t0 + T_LOC].T.astype(gemm_np)          # [K, T_LOC]
        xb = np.ascontiguousarray(
            xt.reshape(KT, 128, NB, TB).transpose(2, 1, 0, 3)
        )
        # warm-up columns: previous chunk's tail (zeros for the first chunk)
        if p == 0:
            xw = np.zeros((128, KT, W), gemm_np)
        else:
            xw = np.ascontiguousarray(
                x[t0 - W:t0].T.astype(gemm_np).reshape(KT, 128, W)
                .transpose(1, 0, 2)
            )
        av = np.ascontiguousarray(a_mat[qs].reshape(NG, 128).T)
        in_maps.append({"xb": xb, "xw": xw, "bt": b_sw, "av": av})
    return in_maps


def kernel(x, a_mat, b_mat):
    from concourse import bass_utils

    key = ("nc1", str(GEMM_DT), NSWEEPS)
    if key not in _CACHE:
        _CACHE[key] = _build(1, GEMM_DT, NSWEEPS)
    nc = _CACHE[key]
    in_maps = make_in_maps(x, a_mat, b_mat)
    res = bass_utils.run_bass_kernel_spmd(nc, in_maps, core_ids=list(range(N_CORES)))
    out = np.empty((T, H), np.float32)
    for c in range(N_CORES):
        p, q = divmod(c, QH)
        ht = np.asarray(res.results[c]["ht"])   # [CPC, T_LOC]
        out[p * T_LOC:(p + 1) * T_LOC, q * CPC:(q + 1) * CPC] = ht.T
    return out


# revision 20
# speedup vs baseline: 1.3515x; 1.0676x over previous
"""Trainium2 Bass kernel for nn_DiagSSMBlock: h_t = tanh(a * h_{t-1} + (x @ b)_t).

Strategy (8 NeuronCores, T x H = 2 x 4 sharding => zero cross-core comm):
  - Cores are arranged 2-way along time (chunks of 2048 steps) x 4-way along
    channels (512 each). The diagonal recurrence is per-channel independent,
    and |a| <= sqrt(2/2048) = 0.03125 (glorot) means a time-chunk started
    from a zero carry converges to the true scan within a few steps: each
    chunk is given W=16 warm-up columns (the tail of the previous chunk;
    zeros for the first chunk), so the boundary error is ~|a|^17 ~ 0.
  - This cuts per-core HBM traffic from 21 MiB (H-only sharding: every core
    reads all of x) to ~14 MiB, moving the kernel from DMA-bound to the
    fp16 matmul roofline (4g*16kt*2064col = 132k PE cycles ~ 55us @2.4GHz).
  - GEMM: sT = b^T x with b stationary; host pre-swizzles x and b into the
    exact SBUF layout ([128 part, kt, t] with 16 KiB contiguous per
    partition) so every DMA descriptor is a 16 KiB run (near-peak DMA).
  - The scan is solved by block Gauss-Seidel fixed-point iteration:
        u^0 = tanh(s);  u^m_t = tanh(a * u^{m-1}_{t-1} + s_t)
    Each sweep contracts the error by |a| <= 0.03125; after u0 + 2 sweeps
    the scan error (~3e-5) is far below the fp16 GEMM rounding (~9e-4 of
    output scale), which dominates the error budget. Sweeps are emitted in
    wavefront order so the scan (DVE+ACT) chases the GEMM (PE).
  - Output DMA is issued from the otherwise-idle Pool/GpSimd queue so the
    ACT and SP sequencers stay free for the scan and input prefetch.
"""

import numpy as np

import jax
from jax.sharding import Mesh, NamedSharding, PartitionSpec
from jax.experimental.shard_map import shard_map

import concourse.tile as tile
from concourse import bacc, mybir
from concourse.bass2jax import (
    _bass_exec_p,
    install_neuronx_cc_hook,
    partition_id_tensor,
)

T = 4096          # sequence length
K = 2048          # input features (contraction dim)
H = 2048          # output channels
N_CORES = 8
PT = 2            # T-shards
QH = 4            # H-shards
T_LOC = T // PT   # timesteps per core (graded)
W = 16            # warm-up columns (zero-carry decay: |a|^17 ~ 1e-25)
T_TOT = T_LOC + W
CPC = H // QH     # channels per core
NG = CPC // 128   # channel groups of 128 partitions
KT = K // 128     # k-tiles
TB = 512          # GEMM moving-dim block (one PSUM bank of fp32)
NB = T_LOC // TB  # main t-blocks per core
NSWEEPS = 1       # Gauss-Seidel refinement sweeps after u0 = tanh(s)
                  # (|a|<=0.03125: worst-case scan err a^2 ~ 9.8e-4, on par
                  #  with the fp16 GEMM rounding; total well under the gate)

F32 = mybir.dt.float32
GEMM_DT = mybir.dt.float16    # full-rate matmul; rel err ~9e-4 of scale


UNROLL = 4          # bodies per For_i trip (input buffers alternate via pools)
STAGGERED = False   # staggered sem reset at the back-edge (no full barrier)
OUT_ENGINE = "gpsimd"      # queue for mid-stream output DMAs
TAIL_OUT_ENGINE = "sync"   # queue for the trip-tail output DMAs (HWDGE)


def _build(loop_iters: int, gemm_dt=GEMM_DT, nsweeps: int = NSWEEPS,
           unroll: int = None, staggered: bool = None):
    if unroll is None:
        unroll = UNROLL
    if staggered is None:
        staggered = STAGGERED
    nc = bacc.Bacc(
        "TRN2", target_bir_lowering=False, debug=False, num_devices=N_CORES
    )

    xb_d = nc.dram_tensor("xb", [NB, 128, KT, TB], gemm_dt, kind="ExternalInput").ap()
    xw_d = nc.dram_tensor("xw", [128, KT, W], gemm_dt, kind="ExternalInput").ap()
    bt_d = nc.dram_tensor("bt", [128, KT, CPC], gemm_dt, kind="ExternalInput").ap()
    av_d = nc.dram_tensor("av", [128, NG], F32, kind="ExternalInput").ap()
    ht_d = nc.dram_tensor("ht", [CPC, T_LOC], F32, kind="ExternalOutput").ap()

    ht_r = ht_d.rearrange("(g p) t -> p g t", g=NG)

    # ranges in warm-up-inclusive column space [0, T_TOT); the tail is
    # tapered so the trailing sweep+output pipeline after the last matmul
    # drains on small blocks (shorter For_i barrier tail)
    widths = [W, TB, TB, TB, TB // 2, TB // 4, TB // 4]
    assert sum(widths) == T_TOT
    ranges, pos = [], 0
    for wd in widths:
        ranges.append((pos, pos + wd))
        pos += wd
    NR = len(ranges)

    with tile.TileContext(nc) as tc:
        with (
            tc.tile_pool(name="state", bufs=1) as state,
            tc.tile_pool(name="bp", bufs=2) as bpool,
            tc.tile_pool(name="xp", bufs=1) as xpool,
            tc.tile_pool(name="ap", bufs=2) as apool,
            tc.tile_pool(name="ps", bufs=6, space="PSUM") as psum,
            tc.tile_pool(name="zp", bufs=4) as zpool,
        ):
            # Fixed tile objects. b/a ping-pong across bodies (parity); the
            # x tiles are refilled in place right after the wave that
            # consumes them, one body ahead of their next use, so every
            # body's inputs are resident when its matmuls start (the For_i
            # back-edge barrier then never exposes DMA latency).
            b_tiles = [bpool.tile([128, KT, CPC], gemm_dt, tag="b",
                                  name=f"b{j}") for j in range(2)]
            a_tiles = [apool.tile([128, NG], F32, tag="a", name=f"a{j}")
                       for j in range(2)]
            xs = [xpool.tile([128, KT, W], gemm_dt, tag="xw", name="xw")]
            xs += [xpool.tile([128, KT, TB], gemm_dt, tag=f"x{nb}",
                              name=f"x{nb}") for nb in range(NB - 1)]
            # the last block is ping-ponged by body parity: its refill for
            # the next body has no WAR left by body top, so its DMA never
            # lands on the trip tail (where it would gate the barrier)
            xl_tiles = [xpool.tile([128, KT, TB], gemm_dt, tag=f"xl{j}",
                                   name=f"xl{j}") for j in range(2)]
            x_src = [xw_d] + [xb_d[nb] for nb in range(NB)]
            U = [state.tile([128, T_TOT + 1], F32, tag=f"U{g}", name=f"U{g}")
                 for g in range(NG)]
            sT = [state.tile([128, T_TOT], F32, tag=f"sT{g}", name=f"sT{g}")
                  for g in range(NG)]

            Tanh = mybir.ActivationFunctionType.Tanh
            MUL = mybir.AluOpType.mult
            ADD = mybir.AluOpType.add

            def prologue():
                nc.sync.dma_start(out=b_tiles[0], in_=bt_d)
                nc.sync.dma_start(out=a_tiles[0], in_=av_d)
                for j in range(len(xs)):
                    nc.sync.dma_start(out=xs[j], in_=x_src[j])
                nc.sync.dma_start(out=xl_tiles[0], in_=xb_d[NB - 1])

            def body(it, parity, load_next):
                b_sb = b_tiles[parity]
                a_sb = a_tiles[parity]
                # range ri -> (sbuf tile, column offset within the tile)
                xl = xl_tiles[parity]
                rmap = []
                for ri, (lo, hi) in enumerate(ranges):
                    if lo < W + (NB - 1) * TB:
                        rmap.append((xs[ri], 0))      # 1:1 tile per range
                    else:
                        rmap.append((xl, lo - (W + (NB - 1) * TB)))
                if load_next:
                    nc.sync.dma_start(out=b_tiles[1 - parity], in_=bt_d)
                    nc.sync.dma_start(out=xl_tiles[1 - parity],
                                      in_=xb_d[NB - 1])
                    nc.sync.dma_start(out=a_tiles[1 - parity], in_=av_d)
                for g in range(NG):
                    nc.vector.memset(U[g][:, 0:1], 0.0)

                def gemm_range(ri):
                    lo, hi = ranges[ri]
                    xt, xo = rmap[ri]
                    for g in range(NG):
                        ps = psum.tile([128, hi - lo], F32, tag="ps")
                        for kt in range(KT):
                            nc.tensor.matmul(
                                ps,
                                lhsT=b_sb[:, kt, g * 128:(g + 1) * 128],
                                rhs=xt[:, kt, xo:xo + (hi - lo)],
                                start=(kt == 0),
                                stop=(kt == KT - 1),
                            )
                        nc.vector.tensor_copy(out=sT[g][:, lo:hi], in_=ps)
                        nc.scalar.activation(
                            out=U[g][:, 1 + lo:1 + hi], in_=ps, func=Tanh
                        )
                    if load_next and ri < NR and rmap[ri][0] is not xl and ri < len(x_src):
                        # refill this block for the next body (consumed
                        # above; next body's wave ri is ~5 waves away)
                        nc.sync.dma_start(out=xs[ri], in_=x_src[ri])

                def sweep_range(m, ri):
                    lo, hi = ranges[ri]
                    last = m == nsweeps - 1
                    for g in range(NG):
                        z = zpool.tile(
                            [128, hi - lo], F32, tag="z",
                            name=f"z_{it}_{m}_{ri}_{g}"
                        )
                        nc.vector.scalar_tensor_tensor(
                            out=z,
                            in0=U[g][:, lo:hi],
                            scalar=a_sb[:, g:g + 1],
                            in1=sT[g][:, lo:hi],
                            op0=MUL,
                            op1=ADD,
                        )
                        nc.scalar.activation(
                            out=U[g][:, 1 + lo:1 + hi], in_=z, func=Tanh
                        )
                        if last and ri > 0:
                            # tail ranges' outputs go on an HWDGE queue so
                            # the trip barrier is not gated by the Pool Q7
                            # descriptor-gen drain
                            eng = TAIL_OUT_ENGINE if ri >= NR - 3 else OUT_ENGINE
                            getattr(nc, eng).dma_start(
                                out=ht_r[:, g, lo - W:hi - W],
                                in_=U[g][:, 1 + lo:1 + hi],
                            )

                for wave in range(NR + nsweeps):
                    if wave < NR:
                        gemm_range(wave)
                    for m in range(1, nsweeps + 1):
                        ri = wave - m
                        if 0 <= ri < NR:
                            sweep_range(m - 1, ri)

            assert unroll % 2 == 0
            trips, rem = divmod(loop_iters, unroll)
            prologue()
            if trips <= 1:
                n = loop_iters
                for u in range(n):
                    body(f"i{u}", u % 2, u < n - 1)
            else:
                with tc.For_i(
                    0, trips, 1, hint_engines=(mybir.EngineType.PE,),
                    staggered_reset=staggered,
                ) as i:
                    for u in range(unroll):
                        body(f"u{u}", u % 2, True)
                for r in range(rem):
                    body(f"r{r}", r % 2, r < rem - 1)

    nc.compile()
    return nc


def _build_runner(nc):
    """Reusable jitted shard_map executable for an 8-core SPMD Bass module."""
    install_neuronx_cc_hook()
    partition_name = nc.partition_id_tensor.name if nc.partition_id_tensor else None
    in_names, out_names, out_avals = [], [], []
    for alloc in nc.m.functions[0].allocations:
        if not isinstance(alloc, mybir.MemoryLocationSet):
            continue
        name = alloc.memorylocations[0].name
        if alloc.kind == "ExternalInput":
            if name != partition_name:
                in_names.append(name)
        elif alloc.kind == "ExternalOutput":
            out_names.append(name)
            out_avals.append(
                jax.core.ShapedArray(
                    tuple(alloc.tensor_shape), mybir.dt.np(alloc.dtype)
                )
            )
    n_params = len(in_names)
    n_outs = len(out_avals)
    in_names_all = list(in_names) + list(out_names)
    if partition_name is not None:
        in_names_all.append(partition_name)
    donate = tuple(range(n_params, n_params + n_outs))

    def _bdy(*args):
        operands = list(args)
        if partition_name is not None:
            operands.append(partition_id_tensor())
        return tuple(
            _bass_exec_p.bind(
                *operands,
                out_avals=tuple(out_avals),
                in_names=tuple(in_names_all),
                out_names=tuple(out_names),
                lowering_input_output_aliases=(),
                sim_require_finite=True,
                sim_require_nnan=True,
                nc=nc,
            )
        )

    devices = jax.devices()[:N_CORES]
    mesh = Mesh(np.asarray(devices), ("core",))
    in_specs = (PartitionSpec("core"),) * (n_params + n_outs)
    out_specs = (PartitionSpec("core"),) * len(out_names)
    sharded = jax.jit(
        shard_map(
            _bdy, mesh=mesh, in_specs=in_specs, out_specs=out_specs,
            check_rep=False,
        ),
        donate_argnums=donate,
        keep_unused=True,
    )
    shardng = NamedSharding(mesh, PartitionSpec("core"))
    out_shapes = [
        (N_CORES * a.shape[0], *a.shape[1:]) for a in out_avals
    ]
    out_dtypes = [a.dtype for a in out_avals]

    class Runner:
        def put_inputs(self, in_maps):
            concat = [
                np.concatenate([m[n] for m in in_maps], axis=0) for n in in_names
            ]
            return [jax.device_put(a, shardng) for a in concat]

        def zeros(self):
            return [
                jax.device_put(np.zeros(s, d), shardng)
                for s, d in zip(out_shapes, out_dtypes)
            ]

        def exec_device(self, dev_in, dev_zeros):
            outs = sharded(*dev_in, *dev_zeros)
            jax.block_until_ready(outs)
            return outs

        def fetch(self, outs):
            return {
                name: np.asarray(outs[i]).reshape(N_CORES, -1, *out_avals[i].shape[1:])
                for i, name in enumerate(out_names)
            }

        def __call__(self, dev_in, dev_zeros):
            return self.fetch(self.exec_device(dev_in, dev_zeros))

    return Runner()


_CACHE: dict = {}


def get_compiled(loop_iters=1, gemm_dt=GEMM_DT, nsweeps=NSWEEPS):
    key = (loop_iters, str(gemm_dt), nsweeps)
    if key not in _CACHE:
        nc = _build(loop_iters, gemm_dt, nsweeps)
        _CACHE[key] = (nc, _build_runner(nc))
    return _CACHE[key]


def make_in_maps(x, a_mat, b_mat, gemm_dt=GEMM_DT):
    gemm_np = mybir.dt.np(gemm_dt)
    x = np.ascontiguousarray(np.asarray(x, np.float32))
    a_mat = np.ascontiguousarray(np.asarray(a_mat, np.float32))
    b_mat = np.ascontiguousarray(np.asarray(b_mat, np.float32))
    in_maps = []
    for c in range(N_CORES):
        p, q = divmod(c, QH)
        t0 = p * T_LOC
        qs = slice(q * CPC, (q + 1) * CPC)
        # b: [k, c] -> [p, kt, c] with k = kt*128 + p
        b_sw = np.ascontiguousarray(
            b_mat[:, qs].reshape(KT, 128, CPC).transpose(1, 0, 2)
        ).astype(gemm_np)
        # x main blocks: [t, k] -> [nb, p, kt, tt], t = t0 + nb*TB + tt
        xt = x[t0:t0 + T_LOC].T.astype(gemm_np)          # [K, T_LOC]
        xb = np.ascontiguousarray(
            xt.reshape(KT, 128, NB, TB).transpose(2, 1, 0, 3)
        )
        # warm-up columns: previous chunk's tail (zeros for the first chunk)
        if p == 0:
            xw = np.zeros((128, KT, W), gemm_np)
        else:
            xw = np.ascontiguousarray(
                x[t0 - W:t0].T.astype(gemm_np).reshape(KT, 128, W)
                .transpose(1, 0, 2)
            )
        av = np.ascontiguousarray(a_mat[qs].reshape(NG, 128).T)
        in_maps.append({"xb": xb, "xw": xw, "bt": b_sw, "av": av})
    return in_maps


def kernel(x, a_mat, b_mat):
    from concourse import bass_utils

    key = ("nc1", str(GEMM_DT), NSWEEPS)
    if key not in _CACHE:
        _CACHE[key] = _build(1, GEMM_DT, NSWEEPS)
    nc = _CACHE[key]
    in_maps = make_in_maps(x, a_mat, b_mat)
    res = bass_utils.run_bass_kernel_spmd(nc, in_maps, core_ids=list(range(N_CORES)))
    out = np.empty((T, H), np.float32)
    for c in range(N_CORES):
        p, q = divmod(c, QH)
        ht = np.asarray(res.results[c]["ht"])   # [CPC, T_LOC]
        out[p * T_LOC:(p + 1) * T_LOC, q * CPC:(q + 1) * CPC] = ht.T
    return out
